# revision 7
# baseline (speedup 1.0000x reference)
# Trainium2 Bass kernel for nn_MixtureOfExperts_37237366456694.
#
# Reference computation (B=4096, D=1024, H1=H2=4096, D_OUT=1024, K=8, G_H=512):
#   U[:,k,:] = MLP_k(x)                      (3-layer ReLU MLP per expert)
#   g        = softmax(gate_MLP(x))          (B, K)
#   Q        = cayley(A); B_k = Q[:, k*128:(k+1)*128]
#   V[:,k,:] = U[:,k,:] @ (B_k B_k^T)
#   out      = (sum_k g[:,k] * V[:,k,:]) @ Wo + bo
#
# Key algebraic collapse (exact):
#   out[b] = sum_k g[b,k] * (U[b,k,:] @ w_k) + bo,   w_k = B_k B_k^T Wo
#          = sum_k g[b,k] * (h2_k[b] @ v_k + c_k) + bo
#   with v_k = W3_k @ w_k  (H2-vector), c_k = b3_k . w_k  (scalar).
# So the third expert layer + subspace projection + output head reduce to a
# dot against a precomputed vector.  The tiny Cayley solve / folds are done
# on host in float64; the heavy compute (two 4096-wide matmul layers) runs
# on device in fp16 with f32 PSUM accumulation.
#
# Sharding: expert-parallel — core k owns expert k (its W1/W2/b1/b2/v shards).
# The kernel is PE(tensor-engine)-row-bound, so everything not a dense matmul
# is pushed off the PE:
#   * v-dot: ACT evicts h2 = relu(psum+b2); DVE (idle) computes t = h2*v
#     with per-partition signed v; GPSIMD (idle) partition-reduces t to a
#     (1,NT) partial per (fc,n) which is DMA'd out; host sums the HC partials.
#     (Replaces per-chunk PE matvecs of the previous version.)
#   * gate: batch-sharded across the 8 cores — core k computes the gate
#     hidden layer + ALL-expert logits for its 1/8 slice of the batch only
#     (the host combines, so no collective is needed).  Replaces an 8x
#     replicated full-batch gate.
# Host combines:  out = (sum_k e_k*(s_k + c_k)) / (sum_k e_k) + bo  with
# e_k = exp(logit_k + bg2_k) in float64 — the softmax normalizer is the
# cross-expert sum, i.e. the "all-reduce" term, done on host for free.
import os

import numpy as np

P = 128


class _Cfg:
    def __init__(self, B=4096, D=1024, H=4096, GH=512, NT=512, SLAB=1024,
                 reps=1):
        self.B, self.D, self.H, self.GH, self.NT, self.SLAB = B, D, H, GH, NT, SLAB
        self.DC = D // P      # d_in chunks
        self.HC = H // P      # hidden chunks (H1 == H2)
        self.GC = GH // P     # gate hidden chunks
        self.NSLAB = B // SLAB
        self.SN = SLAB // NT  # n-tiles per slab
        self.GB = B // 8      # per-core gate batch slice
        self.reps = reps      # >1 only for differential benchmarking


def _build_nc(cfg):
    import contextlib

    import concourse.bass as bass  # noqa: F401
    import concourse.mybir as mybir
    import concourse.tile as tile
    from concourse import bacc

    fp16 = mybir.dt.float16
    f32 = mybir.dt.float32
    Relu = mybir.ActivationFunctionType.Relu
    Add = mybir.AluOpType.add
    AxC = mybir.AxisListType.C

    B, DC, HC, GC, NT, SLAB, SN, NSLAB, GB = (
        cfg.B, cfg.DC, cfg.HC, cfg.GC, cfg.NT, cfg.SLAB, cfg.SN, cfg.NSLAB,
        cfg.GB)
    GH = cfg.GH
    NE = 8  # experts (== cores)

    nc = bacc.Bacc(None, target_bir_lowering=False)
    # [p, dc, b] = x[b, dc*P+p]
    xTd = nc.dram_tensor("xT", (P, DC, B), fp16, kind="ExternalInput")
    # per-core gate slice: [p, dc, b'] = x[core*GB + b', dc*P+p]
    xGd = nc.dram_tensor("xG", (P, DC, GB), fp16, kind="ExternalInput")
    # [hc, p, dc, m] = W1[dc*P+p, hc*P+m]
    W1d = nc.dram_tensor("W1", (HC, P, DC, P), fp16, kind="ExternalInput")
    # [fc, p, hc, m] = W2[hc*P+p, fc*P+m]
    W2d = nc.dram_tensor("W2", (HC, P, HC, P), fp16, kind="ExternalInput")
    # f32 consts: [b1 (HC) | b2 (HC) | bg1 (GC) | v (HC)]
    NF = 3 * HC + GC
    cfd = nc.dram_tensor("constf", (P, NF, 1), f32, kind="ExternalInput")
    # fp16 consts: [Wg2 all-expert columns (GC x NE)]
    NH = GC * NE
    chd = nc.dram_tensor("consth", (P, NH, 1), fp16, kind="ExternalInput")
    # [p, dc, gh] = Wg1[dc*P+p, gh]
    Wg1d = nc.dram_tensor("Wg1", (P, DC, GH), fp16, kind="ExternalInput")
    # per-(fc,n) partial sums of v.h2; host sums over fc
    out_s = nc.dram_tensor("out_s", (HC, B), f32, kind="ExternalOutput")
    # raw gate logits for ALL experts, this core's batch slice
    out_e = nc.dram_tensor("out_e", (NE, GB), f32, kind="ExternalOutput")

    with tile.TileContext(nc) as tc:
        with (
            tc.tile_pool(name="const", bufs=1) as const,
            tc.tile_pool(name="xp", bufs=2) as xp,
            tc.tile_pool(name="zp", bufs=1) as zp,
            tc.tile_pool(name="w1p", bufs=4) as w1p,
            tc.tile_pool(name="w2p", bufs=5) as w2p,
            tc.tile_pool(name="h1p", bufs=1) as h1p,
            tc.tile_pool(name="h2p", bufs=4) as h2p,
            tc.tile_pool(name="tp", bufs=4) as tp,
            tc.tile_pool(name="outp", bufs=4) as outp,
            tc.tile_pool(name="mmps", bufs=6, space="PSUM") as mmps,
            tc.tile_pool(name="vps", bufs=2, space="PSUM") as vps,
        ):
            # --- constants resident in SBUF for the whole kernel ---
            wg1_t = const.tile((P, DC, GH), fp16)
            nc.sync.dma_start(wg1_t[:], Wg1d[:])
            xg_t = const.tile((P, DC, GB), fp16)
            nc.sync.dma_start(xg_t[:], xGd[:])
            cf_t = const.tile((P, NF, 1), f32)
            nc.sync.dma_start(cf_t[:], cfd[:])
            ch_t = const.tile((P, NH, 1), fp16)
            nc.sync.dma_start(ch_t[:], chd[:])
            b1_t = cf_t[:, 0:HC, :]
            b2_t = cf_t[:, HC:2 * HC, :]
            bg1_t = cf_t[:, 2 * HC:2 * HC + GC, :]
            v_t = cf_t[:, 2 * HC + GC:3 * HC + GC, :]

            # reps>1: wrap the computation in a hardware loop.  Every
            # iteration is instruction-identical (same addresses), so this
            # multiplies device time by reps while keeping the NEFF small —
            # used by test.py for differential timing.
            loop_cm = tc.For_i(0, cfg.reps) if cfg.reps > 1 else (
                contextlib.nullcontext())
            with loop_cm:
              # --- gate MLP on this core's batch slice, all experts ---
              z1 = zp.tile((P, GC, GB), fp16, name="z1", tag="z1")
              for gc in range(GC):
                  ps = mmps.tile((P, GB), f32, name="ps_g", tag="mm")
                  for dc in range(DC):
                      nc.tensor.matmul(
                          ps, wg1_t[:, dc, gc * P:(gc + 1) * P],
                          xg_t[:, dc, :],
                          start=(dc == 0), stop=(dc == DC - 1))
                  nc.scalar.activation(z1[:, gc, :], ps, Relu,
                                       bias=bg1_t[:, gc, :])
              lg = vps.tile((NE, GB), f32, name="lg", tag="vec")
              for gc in range(GC):
                  nc.tensor.matmul(
                      lg, ch_t[:, gc * NE:(gc + 1) * NE, :],
                      z1[:, gc, :],
                      start=(gc == 0), stop=(gc == GC - 1))
              lt = outp.tile((NE, GB), f32, name="lt", tag="lt")
              nc.vector.tensor_copy(lt[:], lg)
              nc.sync.dma_start(out_e[:, :], lt[:])

              for sl in range(NSLAB):
                s0 = sl * SLAB
                # --- x slab (transposed: d on partitions) ---
                xt = xp.tile((P, DC, SLAB), fp16, name="xt", tag="xt")
                nc.sync.dma_start(xt[:], xTd[:, :, s0:s0 + SLAB])

                # --- layer 1: h1 = relu(x @ W1 + b1), stored transposed ---
                h1 = h1p.tile((P, HC, SLAB), fp16, name="h1", tag="h1")
                for hc in range(HC):
                    w1s = w1p.tile((P, DC, P), fp16, name="w1s", tag="w1s")
                    nc.sync.dma_start(w1s[:], W1d[hc])
                    for n in range(SN):
                        ns = slice(n * NT, (n + 1) * NT)
                        ps = mmps.tile((P, NT), f32, name="ps_1", tag="mm")
                        for dc in range(DC):
                            nc.tensor.matmul(ps, w1s[:, dc, :], xt[:, dc, ns],
                                             start=(dc == 0),
                                             stop=(dc == DC - 1))
                        nc.scalar.activation(h1[:, hc, ns], ps, Relu,
                                             bias=b1_t[:, hc, :])

                # --- layer 2 + v-dot partials (ACT -> DVE -> GPSIMD) ---
                for fc in range(HC):
                    w2s = w2p.tile((P, HC, P), fp16, name="w2s", tag="w2s")
                    nc.sync.dma_start(w2s[:], W2d[fc])
                    for n in range(SN):
                        ns = slice(n * NT, (n + 1) * NT)
                        ps = mmps.tile((P, NT), f32, name="ps_2", tag="mm")
                        for hc in range(HC):
                            nc.tensor.matmul(ps, w2s[:, hc, :], h1[:, hc, ns],
                                             start=(hc == 0),
                                             stop=(hc == HC - 1))
                        h2t = h2p.tile((P, NT), fp16, name="h2t", tag="h2t")
                        nc.scalar.activation(h2t[:], ps, Relu,
                                             bias=b2_t[:, fc, :])
                        tt = tp.tile((P, NT), f32, name="tt", tag="tt")
                        nc.vector.tensor_scalar_mul(tt[:], h2t[:],
                                                    v_t[:, fc, :])
                        ot = outp.tile((1, NT), f32, name="ot", tag="ot")
                        nc.gpsimd.tensor_reduce(ot[:], tt[:], AxC, Add)
                        nc.sync.dma_start(
                            out_s[fc:fc + 1, s0 + n * NT:s0 + (n + 1) * NT],
                            ot[:])
    nc.compile()
    return nc


_STATE = {}
LAST_RESULTS = None  # BassKernelResults of the most recent device run
LAST_RUN_SECONDS = None  # wall time of the device-run call (excl. host prep)


def _get_nc(cfg):
    key = (cfg.B, cfg.D, cfg.H, cfg.GH, cfg.NT, cfg.SLAB, cfg.reps)
    if key not in _STATE:
        _STATE[key] = _build_nc(cfg)
    return _STATE[key]


def _fold(W3, b3, A, Wo):
    """v_k = W3_k @ (B_k B_k^T Wo),  c_k = b3_k . (B_k B_k^T Wo)  in float64."""
    A64 = A.astype(np.float64)
    S = A64 - A64.T
    I = np.eye(A.shape[0])
    Q = np.linalg.solve(I - S, I + S)
    K = W3.shape[0]
    sub = Q.shape[1] // K
    Bq = Q.reshape(Q.shape[0], K, sub)                      # [d, k, s]
    coef = np.einsum('dks,d->ks', Bq, Wo[:, 0].astype(np.float64))
    w = np.einsum('dks,ks->kd', Bq, coef)                   # (K, dim)
    v = np.einsum('kfd,kd->kf', W3.astype(np.float64), w)   # (K, H2)
    c = np.einsum('kd,kd->k', b3.astype(np.float64), w)     # (K,)
    return v, c


def _prep_in_maps(cfg, x, W1, b1, W2, b2, v, Wg1, bg1, Wg2, bg2):
    fp16 = np.float16
    f32 = np.float32
    K = W1.shape[0]
    DC, HC, GC, GB = cfg.DC, cfg.HC, cfg.GC, cfg.GB
    NE = K

    # [p, dc, b]
    xT = np.ascontiguousarray(
        x.astype(fp16).T.reshape(DC, P, cfg.B).transpose(1, 0, 2))
    W1p = np.ascontiguousarray(
        W1.astype(fp16).reshape(K, DC, P, HC, P).transpose(0, 3, 2, 1, 4))
    W2p = np.ascontiguousarray(
        W2.astype(fp16).reshape(K, HC, P, HC, P).transpose(0, 3, 2, 1, 4))
    Wg1p = np.ascontiguousarray(
        Wg1.astype(fp16).reshape(DC, P, cfg.GH).transpose(1, 0, 2))

    # packed f32 consts (P, 3*HC+GC, 1): [b1 | b2 | bg1 | v]
    NF = 3 * HC + GC
    constf = np.empty((K, P, NF, 1), f32)
    constf[:, :, 0:HC, 0] = b1.astype(f32).reshape(K, HC, P).transpose(0, 2, 1)
    constf[:, :, HC:2 * HC, 0] = (
        b2.astype(f32).reshape(K, HC, P).transpose(0, 2, 1))
    constf[:, :, 2 * HC:2 * HC + GC, 0] = (
        bg1.astype(f32).reshape(GC, P).T[None])
    constf[:, :, 2 * HC + GC:, 0] = (
        v.astype(f32).reshape(K, HC, P).transpose(0, 2, 1))
    # packed fp16 consts (P, GC*NE, 1): [Wg2 all columns]
    NH = GC * NE
    consth = np.empty((K, P, NH, 1), fp16)
    # [p, gc*NE + e] = Wg2[gc*P+p, e] — same for every core
    wg2_packed = (
        Wg2.astype(fp16).reshape(GC, P, NE).transpose(1, 0, 2).reshape(
            P, GC * NE))
    consth[:, :, :, 0] = wg2_packed[None]

    in_maps = []
    for k in range(K):
        in_maps.append({
            "xT": xT,
            "xG": np.ascontiguousarray(xT[:, :, k * GB:(k + 1) * GB]),
            "W1": W1p[k],
            "W2": W2p[k],
            "constf": constf[k],
            "consth": consth[k],
            "Wg1": Wg1p,
        })
    return in_maps


def kernel(x, W1, b1, W2, b2, W3, b3, Wg1, bg1, Wg2, bg2, A, Wo, bo):
    global LAST_RESULTS, LAST_RUN_SECONDS
    import time

    from concourse.bass_utils import run_bass_kernel_spmd

    cfg = _Cfg(B=x.shape[0], D=x.shape[1], H=W1.shape[2], GH=Wg1.shape[1])
    K = W1.shape[0]

    v, c = _fold(W3, b3, A, Wo)
    in_maps = _prep_in_maps(cfg, x, W1, b1, W2, b2, v, Wg1, bg1, Wg2, bg2)
    nc = _get_nc(cfg)

    trace = bool(int(os.environ.get("MOE_TRACE", "0")))
    t0 = time.time()
    res = run_bass_kernel_spmd(
        nc, in_maps, core_ids=list(range(K)), trace=trace,
        trace_cores=list(range(K)) if trace else None,
    )
    LAST_RUN_SECONDS = time.time() - t0
    LAST_RESULTS = res

    GB = cfg.GB
    # s_k[b] = sum_fc partial[fc, b]  (v already applied, signed, on device)
    s = np.stack([r["out_s"].astype(np.float64).sum(axis=0)
                  for r in res.results])
    # logits: core k supplies all-expert logits for batch slice k
    logit = np.concatenate([r["out_e"] for r in res.results],
                           axis=1).astype(np.float64)
    e = np.exp(logit + bg2.astype(np.float64)[:, None])
    num = (e * (s + c[:, None])).sum(axis=0)
    den = e.sum(axis=0)
    out = num / den + float(bo[0])
    return out.astype(np.float32)[:, None]


# revision 8
# speedup vs baseline: 6.1302x; 6.1302x over previous
# Trainium2 Bass kernel for nn_MixtureOfExperts_37237366456694.
#
# Reference computation (B=4096, D=1024, H1=H2=4096, D_OUT=1024, K=8, G_H=512):
#   U[:,k,:] = MLP_k(x)                      (3-layer ReLU MLP per expert)
#   g        = softmax(gate_MLP(x))          (B, K)
#   Q        = cayley(A); B_k = Q[:, k*128:(k+1)*128]
#   V[:,k,:] = U[:,k,:] @ (B_k B_k^T)
#   out      = (sum_k g[:,k] * V[:,k,:]) @ Wo + bo
#
# Key algebraic collapse (exact):
#   out[b] = sum_k g[b,k] * (U[b,k,:] @ w_k) + bo,   w_k = B_k B_k^T Wo
#          = sum_k g[b,k] * (h2_k[b] @ v_k + c_k) + bo
#   with v_k = W3_k @ w_k  (H2-vector), c_k = b3_k . w_k  (scalar).
# So the third expert layer + subspace projection + output head reduce to a
# dot against a precomputed vector.  The tiny Cayley solve / folds are done
# on host in float64; the heavy compute (two 4096-wide matmul layers) runs
# on device in fp16 with f32 PSUM accumulation.
#
# Sharding: expert-parallel — core k owns expert k (its W1/W2/b1/b2/v shards).
# The kernel is PE(tensor-engine)-row-bound, so everything not a dense matmul
# is pushed off the PE:
#   * v-dot: ACT evicts h2 = relu(psum+b2); DVE (idle) accumulates
#     facc += h2*v in f32 with per-partition signed v across the fc chunks;
#     one 512-row fp32 PE matvec against a ones vector per (slab,n) reduces
#     facc over partitions at slab end (~7us/rep on the PE instead of the
#     ~66us/rep of per-chunk PE matvecs).
#   * gate: batch-sharded across the 8 cores — core k computes the gate
#     hidden layer + ALL-expert logits for its 1/8 slice of the batch only
#     (the host combines, so no collective is needed).  Replaces an 8x
#     replicated full-batch gate.
# Host combines:  out = (sum_k e_k*(s_k + c_k)) / (sum_k e_k) + bo  with
# e_k = exp(logit_k + bg2_k) in float64 — the softmax normalizer is the
# cross-expert sum, i.e. the "all-reduce" term, done on host for free.
import os

import numpy as np

P = 128


class _Cfg:
    def __init__(self, B=4096, D=1024, H=4096, GH=512, NT=512, SLAB=1024,
                 reps=1):
        self.B, self.D, self.H, self.GH, self.NT, self.SLAB = B, D, H, GH, NT, SLAB
        self.DC = D // P      # d_in chunks
        self.HC = H // P      # hidden chunks (H1 == H2)
        self.GC = GH // P     # gate hidden chunks
        self.NSLAB = B // SLAB
        self.SN = SLAB // NT  # n-tiles per slab
        self.GB = B // 8      # per-core gate batch slice
        self.reps = reps      # >1 only for differential benchmarking


def _build_nc(cfg):
    import contextlib

    import concourse.bass as bass  # noqa: F401
    import concourse.mybir as mybir
    import concourse.tile as tile
    from concourse import bacc

    fp16 = mybir.dt.float16
    f32 = mybir.dt.float32
    Relu = mybir.ActivationFunctionType.Relu
    Add = mybir.AluOpType.add
    AxC = mybir.AxisListType.C

    B, DC, HC, GC, NT, SLAB, SN, NSLAB, GB = (
        cfg.B, cfg.DC, cfg.HC, cfg.GC, cfg.NT, cfg.SLAB, cfg.SN, cfg.NSLAB,
        cfg.GB)
    GH = cfg.GH
    NE = 8  # experts (== cores)

    nc = bacc.Bacc(None, target_bir_lowering=False)
    # [p, dc, b] = x[b, dc*P+p]
    xTd = nc.dram_tensor("xT", (P, DC, B), fp16, kind="ExternalInput")
    # per-core gate slice: [p, dc, b'] = x[core*GB + b', dc*P+p]
    xGd = nc.dram_tensor("xG", (P, DC, GB), fp16, kind="ExternalInput")
    # [hc, p, dc, m] = W1[dc*P+p, hc*P+m]
    W1d = nc.dram_tensor("W1", (HC, P, DC, P), fp16, kind="ExternalInput")
    # [fc, p, hc, m] = W2[hc*P+p, fc*P+m]
    W2d = nc.dram_tensor("W2", (HC, P, HC, P), fp16, kind="ExternalInput")
    # f32 consts: [b1 (HC) | b2 (HC) | bg1 (GC) | v (HC)]
    NF = 3 * HC + GC
    cfd = nc.dram_tensor("constf", (P, NF, 1), f32, kind="ExternalInput")
    # fp16 consts: [Wg2 all-expert columns (GC x NE)]
    NH = GC * NE
    chd = nc.dram_tensor("consth", (P, NH, 1), fp16, kind="ExternalInput")
    # [p, dc, gh] = Wg1[dc*P+p, gh]
    Wg1d = nc.dram_tensor("Wg1", (P, DC, GH), fp16, kind="ExternalInput")
    out_s = nc.dram_tensor("out_s", (1, B), f32, kind="ExternalOutput")
    # raw gate logits for ALL experts, this core's batch slice
    out_e = nc.dram_tensor("out_e", (NE, GB), f32, kind="ExternalOutput")

    with tile.TileContext(nc) as tc:
        with (
            tc.tile_pool(name="const", bufs=1) as const,
            tc.tile_pool(name="xp", bufs=2) as xp,
            tc.tile_pool(name="zp", bufs=1) as zp,
            tc.tile_pool(name="w1p", bufs=4) as w1p,
            tc.tile_pool(name="w2p", bufs=5) as w2p,
            tc.tile_pool(name="h1p", bufs=1) as h1p,
            tc.tile_pool(name="h2p", bufs=4) as h2p,
            tc.tile_pool(name="tp", bufs=3) as tp,
            tc.tile_pool(name="fbp", bufs=3) as fbp,
            tc.tile_pool(name="outp", bufs=4) as outp,
            tc.tile_pool(name="mmps", bufs=6, space="PSUM") as mmps,
            tc.tile_pool(name="vps", bufs=2, space="PSUM") as vps,
        ):
            # --- constants resident in SBUF for the whole kernel ---
            wg1_t = const.tile((P, DC, GH), fp16)
            nc.sync.dma_start(wg1_t[:], Wg1d[:])
            xg_t = const.tile((P, DC, GB), fp16)
            nc.sync.dma_start(xg_t[:], xGd[:])
            cf_t = const.tile((P, NF, 1), f32)
            nc.sync.dma_start(cf_t[:], cfd[:])
            ch_t = const.tile((P, NH, 1), fp16)
            nc.sync.dma_start(ch_t[:], chd[:])
            ones_t = const.tile((P, 1), f32)
            nc.vector.memset(ones_t[:], 1.0)
            b1_t = cf_t[:, 0:HC, :]
            b2_t = cf_t[:, HC:2 * HC, :]
            bg1_t = cf_t[:, 2 * HC:2 * HC + GC, :]
            v_t = cf_t[:, 2 * HC + GC:3 * HC + GC, :]

            # reps>1: wrap the computation in a hardware loop.  Every
            # iteration is instruction-identical (same addresses), so this
            # multiplies device time by reps while keeping the NEFF small —
            # used by test.py for differential timing.
            loop_cm = tc.For_i(0, cfg.reps) if cfg.reps > 1 else (
                contextlib.nullcontext())
            with loop_cm:
              # --- gate MLP on this core's batch slice, all experts ---
              z1 = zp.tile((P, GC, GB), fp16, name="z1", tag="z1")
              for gc in range(GC):
                  ps = mmps.tile((P, GB), f32, name="ps_g", tag="mm")
                  for dc in range(DC):
                      nc.tensor.matmul(
                          ps, wg1_t[:, dc, gc * P:(gc + 1) * P],
                          xg_t[:, dc, :],
                          start=(dc == 0), stop=(dc == DC - 1))
                  nc.scalar.activation(z1[:, gc, :], ps, Relu,
                                       bias=bg1_t[:, gc, :])
              lg = vps.tile((NE, GB), f32, name="lg", tag="vec")
              for gc in range(GC):
                  nc.tensor.matmul(
                      lg, ch_t[:, gc * NE:(gc + 1) * NE, :],
                      z1[:, gc, :],
                      start=(gc == 0), stop=(gc == GC - 1))
              lt = outp.tile((NE, GB), f32, name="lt", tag="lt")
              nc.vector.tensor_copy(lt[:], lg)
              nc.sync.dma_start(out_e[:, :], lt[:])

              for sl in range(NSLAB):
                s0 = sl * SLAB
                # --- x slab (transposed: d on partitions) ---
                xt = xp.tile((P, DC, SLAB), fp16, name="xt", tag="xt")
                nc.sync.dma_start(xt[:], xTd[:, :, s0:s0 + SLAB])

                # --- layer 1: h1 = relu(x @ W1 + b1), stored transposed ---
                h1 = h1p.tile((P, HC, SLAB), fp16, name="h1", tag="h1")
                for hc in range(HC):
                    w1s = w1p.tile((P, DC, P), fp16, name="w1s", tag="w1s")
                    nc.sync.dma_start(w1s[:], W1d[hc])
                    for n in range(SN):
                        ns = slice(n * NT, (n + 1) * NT)
                        ps = mmps.tile((P, NT), f32, name="ps_1", tag="mm")
                        for dc in range(DC):
                            nc.tensor.matmul(ps, w1s[:, dc, :], xt[:, dc, ns],
                                             start=(dc == 0),
                                             stop=(dc == DC - 1))
                        nc.scalar.activation(h1[:, hc, ns], ps, Relu,
                                             bias=b1_t[:, hc, :])

                # --- layer 2 + v-dot (ACT -> DVE accumulate) ---
                facc = [fbp.tile((P, NT), f32, name=f"facc{n}", tag="facc")
                        for n in range(SN)]
                for fc in range(HC):
                    w2s = w2p.tile((P, HC, P), fp16, name="w2s", tag="w2s")
                    nc.sync.dma_start(w2s[:], W2d[fc])
                    for n in range(SN):
                        ns = slice(n * NT, (n + 1) * NT)
                        ps = mmps.tile((P, NT), f32, name="ps_2", tag="mm")
                        for hc in range(HC):
                            nc.tensor.matmul(ps, w2s[:, hc, :], h1[:, hc, ns],
                                             start=(hc == 0),
                                             stop=(hc == HC - 1))
                        h2t = h2p.tile((P, NT), fp16, name="h2t", tag="h2t")
                        nc.scalar.activation(h2t[:], ps, Relu,
                                             bias=b2_t[:, fc, :])
                        if fc == 0:
                            nc.vector.tensor_scalar_mul(facc[n][:], h2t[:],
                                                        v_t[:, fc, :])
                        else:
                            tt = tp.tile((P, NT), f32, name="tt", tag="tt")
                            nc.vector.tensor_scalar_mul(tt[:], h2t[:],
                                                        v_t[:, fc, :])
                            nc.vector.tensor_tensor(facc[n][:], facc[n][:],
                                                    tt[:], Add)
                # partition-reduce facc via a single fp32 ones-matvec per n
                for n in range(SN):
                    sps = vps.tile((1, NT), f32, name="sps", tag="vec")
                    nc.tensor.matmul(sps, ones_t[:], facc[n][:],
                                     start=True, stop=True)
                    ot = outp.tile((1, NT), f32, name="ot", tag="ot")
                    nc.vector.tensor_copy(ot[:], sps)
                    nc.sync.dma_start(
                        out_s[0:1, s0 + n * NT:s0 + (n + 1) * NT], ot[:])
    nc.compile()
    return nc


_STATE = {}
LAST_RESULTS = None  # BassKernelResults of the most recent device run
LAST_RUN_SECONDS = None  # wall time of the device-run call (excl. host prep)


def _get_nc(cfg):
    key = (cfg.B, cfg.D, cfg.H, cfg.GH, cfg.NT, cfg.SLAB, cfg.reps)
    if key not in _STATE:
        _STATE[key] = _build_nc(cfg)
    return _STATE[key]


def _fold(W3, b3, A, Wo):
    """v_k = W3_k @ (B_k B_k^T Wo),  c_k = b3_k . (B_k B_k^T Wo)  in float64."""
    A64 = A.astype(np.float64)
    S = A64 - A64.T
    I = np.eye(A.shape[0])
    Q = np.linalg.solve(I - S, I + S)
    K = W3.shape[0]
    sub = Q.shape[1] // K
    Bq = Q.reshape(Q.shape[0], K, sub)                      # [d, k, s]
    coef = np.einsum('dks,d->ks', Bq, Wo[:, 0].astype(np.float64))
    w = np.einsum('dks,ks->kd', Bq, coef)                   # (K, dim)
    v = np.einsum('kfd,kd->kf', W3.astype(np.float64), w)   # (K, H2)
    c = np.einsum('kd,kd->k', b3.astype(np.float64), w)     # (K,)
    return v, c


def _prep_in_maps(cfg, x, W1, b1, W2, b2, v, Wg1, bg1, Wg2, bg2):
    fp16 = np.float16
    f32 = np.float32
    K = W1.shape[0]
    DC, HC, GC, GB = cfg.DC, cfg.HC, cfg.GC, cfg.GB
    NE = K

    # [p, dc, b]
    xT = np.ascontiguousarray(
        x.astype(fp16).T.reshape(DC, P, cfg.B).transpose(1, 0, 2))
    W1p = np.ascontiguousarray(
        W1.astype(fp16).reshape(K, DC, P, HC, P).transpose(0, 3, 2, 1, 4))
    W2p = np.ascontiguousarray(
        W2.astype(fp16).reshape(K, HC, P, HC, P).transpose(0, 3, 2, 1, 4))
    Wg1p = np.ascontiguousarray(
        Wg1.astype(fp16).reshape(DC, P, cfg.GH).transpose(1, 0, 2))

    # packed f32 consts (P, 3*HC+GC, 1): [b1 | b2 | bg1 | v]
    NF = 3 * HC + GC
    constf = np.empty((K, P, NF, 1), f32)
    constf[:, :, 0:HC, 0] = b1.astype(f32).reshape(K, HC, P).transpose(0, 2, 1)
    constf[:, :, HC:2 * HC, 0] = (
        b2.astype(f32).reshape(K, HC, P).transpose(0, 2, 1))
    constf[:, :, 2 * HC:2 * HC + GC, 0] = (
        bg1.astype(f32).reshape(GC, P).T[None])
    constf[:, :, 2 * HC + GC:, 0] = (
        v.astype(f32).reshape(K, HC, P).transpose(0, 2, 1))
    # packed fp16 consts (P, GC*NE, 1): [Wg2 all columns]
    NH = GC * NE
    consth = np.empty((K, P, NH, 1), fp16)
    # [p, gc*NE + e] = Wg2[gc*P+p, e] — same for every core
    wg2_packed = (
        Wg2.astype(fp16).reshape(GC, P, NE).transpose(1, 0, 2).reshape(
            P, GC * NE))
    consth[:, :, :, 0] = wg2_packed[None]

    in_maps = []
    for k in range(K):
        in_maps.append({
            "xT": xT,
            "xG": np.ascontiguousarray(xT[:, :, k * GB:(k + 1) * GB]),
            "W1": W1p[k],
            "W2": W2p[k],
            "constf": constf[k],
            "consth": consth[k],
            "Wg1": Wg1p,
        })
    return in_maps


def kernel(x, W1, b1, W2, b2, W3, b3, Wg1, bg1, Wg2, bg2, A, Wo, bo):
    global LAST_RESULTS, LAST_RUN_SECONDS
    import time

    from concourse.bass_utils import run_bass_kernel_spmd

    cfg = _Cfg(B=x.shape[0], D=x.shape[1], H=W1.shape[2], GH=Wg1.shape[1])
    K = W1.shape[0]

    v, c = _fold(W3, b3, A, Wo)
    in_maps = _prep_in_maps(cfg, x, W1, b1, W2, b2, v, Wg1, bg1, Wg2, bg2)
    nc = _get_nc(cfg)

    trace = bool(int(os.environ.get("MOE_TRACE", "0")))
    t0 = time.time()
    res = run_bass_kernel_spmd(
        nc, in_maps, core_ids=list(range(K)), trace=trace,
        trace_cores=list(range(K)) if trace else None,
    )
    LAST_RUN_SECONDS = time.time() - t0
    LAST_RESULTS = res

    GB = cfg.GB
    s = np.stack([r["out_s"][0] for r in res.results]).astype(np.float64)
    # logits: core k supplies all-expert logits for batch slice k
    logit = np.concatenate([r["out_e"] for r in res.results],
                           axis=1).astype(np.float64)
    e = np.exp(logit + bg2.astype(np.float64)[:, None])
    num = (e * (s + c[:, None])).sum(axis=0)
    den = e.sum(axis=0)
    out = num / den + float(bo[0])
    return out.astype(np.float32)[:, None]


# revision 10
# speedup vs baseline: 6.1775x; 1.0077x over previous
# Trainium2 Bass kernel for nn_MixtureOfExperts_37237366456694.
#
# Reference computation (B=4096, D=1024, H1=H2=4096, D_OUT=1024, K=8, G_H=512):
#   U[:,k,:] = MLP_k(x)                      (3-layer ReLU MLP per expert)
#   g        = softmax(gate_MLP(x))          (B, K)
#   Q        = cayley(A); B_k = Q[:, k*128:(k+1)*128]
#   V[:,k,:] = U[:,k,:] @ (B_k B_k^T)
#   out      = (sum_k g[:,k] * V[:,k,:]) @ Wo + bo
#
# Key algebraic collapse (exact):
#   out[b] = sum_k g[b,k] * (U[b,k,:] @ w_k) + bo,   w_k = B_k B_k^T Wo
#          = sum_k g[b,k] * (h2_k[b] @ v_k + c_k) + bo
#   with v_k = W3_k @ w_k  (H2-vector), c_k = b3_k . w_k  (scalar).
# So the third expert layer + subspace projection + output head reduce to a
# dot against a precomputed vector.  The tiny Cayley solve / folds are done
# on host in float64; the heavy compute (two 4096-wide matmul layers) runs
# on device in fp16 with f32 PSUM accumulation.
#
# Sharding: expert-parallel — core k owns expert k (its W1/W2/b1/b2/v shards).
# The kernel is PE(tensor-engine)-row-bound, so everything not a dense matmul
# is pushed off the PE:
#   * v-dot: ACT evicts h2 = relu(psum+b2); DVE (idle) accumulates
#     facc += h2*v in f32 with per-partition signed v across the fc chunks;
#     one 512-row fp32 PE matvec against a ones vector per (slab,n) reduces
#     facc over partitions at slab end (~7us/rep on the PE instead of the
#     ~66us/rep of per-chunk PE matvecs).
#   * gate: batch-sharded across the 8 cores — core k computes the gate
#     hidden layer + ALL-expert logits for its 1/8 slice of the batch only
#     (the host combines, so no collective is needed).  Replaces an 8x
#     replicated full-batch gate.
# Host combines:  out = (sum_k e_k*(s_k + c_k)) / (sum_k e_k) + bo  with
# e_k = exp(logit_k + bg2_k) in float64 — the softmax normalizer is the
# cross-expert sum, i.e. the "all-reduce" term, done on host for free.
import os

import numpy as np

P = 128


class _Cfg:
    def __init__(self, B=4096, D=1024, H=4096, GH=512, NT=512, SLAB=1024,
                 reps=1):
        self.B, self.D, self.H, self.GH, self.NT, self.SLAB = B, D, H, GH, NT, SLAB
        self.DC = D // P      # d_in chunks
        self.HC = H // P      # hidden chunks (H1 == H2)
        self.GC = GH // P     # gate hidden chunks
        self.NSLAB = B // SLAB
        self.SN = SLAB // NT  # n-tiles per slab
        self.GB = B // 8      # per-core gate batch slice
        self.reps = reps      # >1 only for differential benchmarking


def _build_nc(cfg):
    import contextlib

    import concourse.bass as bass  # noqa: F401
    import concourse.mybir as mybir
    import concourse.tile as tile
    from concourse import bacc

    fp16 = mybir.dt.float16
    f32 = mybir.dt.float32
    Relu = mybir.ActivationFunctionType.Relu
    Add = mybir.AluOpType.add
    AxC = mybir.AxisListType.C

    B, DC, HC, GC, NT, SLAB, SN, NSLAB, GB = (
        cfg.B, cfg.DC, cfg.HC, cfg.GC, cfg.NT, cfg.SLAB, cfg.SN, cfg.NSLAB,
        cfg.GB)
    GH = cfg.GH
    NE = 8  # experts (== cores)

    nc = bacc.Bacc(None, target_bir_lowering=False)
    # [p, dc, b] = x[b, dc*P+p]
    xTd = nc.dram_tensor("xT", (P, DC, B), fp16, kind="ExternalInput")
    # per-core gate slice: [p, dc, b'] = x[core*GB + b', dc*P+p]
    xGd = nc.dram_tensor("xG", (P, DC, GB), fp16, kind="ExternalInput")
    # [hc, p, dc, m] = W1[dc*P+p, hc*P+m]
    W1d = nc.dram_tensor("W1", (HC, P, DC, P), fp16, kind="ExternalInput")
    # [fc, p, hc, m] = W2[hc*P+p, fc*P+m]
    W2d = nc.dram_tensor("W2", (HC, P, HC, P), fp16, kind="ExternalInput")
    # f32 consts: [b1 (HC) | b2 (HC) | bg1 (GC) | v (HC)]
    NF = 3 * HC + GC
    cfd = nc.dram_tensor("constf", (P, NF, 1), f32, kind="ExternalInput")
    # fp16 consts: [Wg2 all-expert columns (GC x NE)]
    NH = GC * NE
    chd = nc.dram_tensor("consth", (P, NH, 1), fp16, kind="ExternalInput")
    # [p, dc, gh] = Wg1[dc*P+p, gh]
    Wg1d = nc.dram_tensor("Wg1", (P, DC, GH), fp16, kind="ExternalInput")
    out_s = nc.dram_tensor("out_s", (1, B), f32, kind="ExternalOutput")
    # raw gate logits for ALL experts, this core's batch slice
    out_e = nc.dram_tensor("out_e", (NE, GB), f32, kind="ExternalOutput")

    with tile.TileContext(nc) as tc:
        with (
            tc.tile_pool(name="const", bufs=1) as const,
            tc.tile_pool(name="xp", bufs=2) as xp,
            tc.tile_pool(name="zp", bufs=1) as zp,
            tc.tile_pool(name="w1p", bufs=4) as w1p,
            tc.tile_pool(name="w2p", bufs=5) as w2p,
            tc.tile_pool(name="h1p", bufs=1) as h1p,
            tc.tile_pool(name="h2p", bufs=4) as h2p,
            tc.tile_pool(name="tp", bufs=3) as tp,
            tc.tile_pool(name="fbp", bufs=SN + 2) as fbp,
            tc.tile_pool(name="outp", bufs=4) as outp,
            tc.tile_pool(name="mmps", bufs=6, space="PSUM") as mmps,
            tc.tile_pool(name="vps", bufs=2, space="PSUM") as vps,
        ):
            # --- constants resident in SBUF for the whole kernel ---
            wg1_t = const.tile((P, DC, GH), fp16)
            nc.sync.dma_start(wg1_t[:], Wg1d[:])
            xg_t = const.tile((P, DC, GB), fp16)
            nc.sync.dma_start(xg_t[:], xGd[:])
            cf_t = const.tile((P, NF, 1), f32)
            nc.sync.dma_start(cf_t[:], cfd[:])
            ch_t = const.tile((P, NH, 1), fp16)
            nc.sync.dma_start(ch_t[:], chd[:])
            ones_t = const.tile((P, 1), f32)
            nc.vector.memset(ones_t[:], 1.0)
            b1_t = cf_t[:, 0:HC, :]
            b2_t = cf_t[:, HC:2 * HC, :]
            bg1_t = cf_t[:, 2 * HC:2 * HC + GC, :]
            v_t = cf_t[:, 2 * HC + GC:3 * HC + GC, :]

            # reps>1: wrap the computation in a hardware loop.  Every
            # iteration is instruction-identical (same addresses), so this
            # multiplies device time by reps while keeping the NEFF small —
            # used by test.py for differential timing.
            loop_cm = tc.For_i(0, cfg.reps) if cfg.reps > 1 else (
                contextlib.nullcontext())
            with loop_cm:
              # --- gate MLP on this core's batch slice, all experts ---
              GN = GB // NT
              z1 = zp.tile((P, GC, GB), fp16, name="z1", tag="z1")
              for gc in range(GC):
                  for gn in range(GN):
                      gs = slice(gn * NT, (gn + 1) * NT)
                      ps = mmps.tile((P, NT), f32, name="ps_g", tag="mm")
                      for dc in range(DC):
                          nc.tensor.matmul(
                              ps, wg1_t[:, dc, gc * P:(gc + 1) * P],
                              xg_t[:, dc, gs],
                              start=(dc == 0), stop=(dc == DC - 1))
                      nc.scalar.activation(z1[:, gc, gs], ps, Relu,
                                           bias=bg1_t[:, gc, :])
              for gn in range(GN):
                  gs = slice(gn * NT, (gn + 1) * NT)
                  lg = vps.tile((NE, NT), f32, name="lg", tag="vec")
                  for gc in range(GC):
                      nc.tensor.matmul(
                          lg, ch_t[:, gc * NE:(gc + 1) * NE, :],
                          z1[:, gc, gs],
                          start=(gc == 0), stop=(gc == GC - 1))
                  lt = outp.tile((NE, NT), f32, name="lt", tag="lt")
                  nc.vector.tensor_copy(lt[:], lg)
                  nc.sync.dma_start(out_e[:, gs], lt[:])

              for sl in range(NSLAB):
                s0 = sl * SLAB
                # --- x slab (transposed: d on partitions) ---
                xt = xp.tile((P, DC, SLAB), fp16, name="xt", tag="xt")
                nc.sync.dma_start(xt[:], xTd[:, :, s0:s0 + SLAB])

                # --- layer 1: h1 = relu(x @ W1 + b1), stored transposed ---
                h1 = h1p.tile((P, HC, SLAB), fp16, name="h1", tag="h1")
                for hc in range(HC):
                    w1s = w1p.tile((P, DC, P), fp16, name="w1s", tag="w1s")
                    nc.sync.dma_start(w1s[:], W1d[hc])
                    for n in range(SN):
                        ns = slice(n * NT, (n + 1) * NT)
                        ps = mmps.tile((P, NT), f32, name="ps_1", tag="mm")
                        for dc in range(DC):
                            nc.tensor.matmul(ps, w1s[:, dc, :], xt[:, dc, ns],
                                             start=(dc == 0),
                                             stop=(dc == DC - 1))
                        nc.scalar.activation(h1[:, hc, ns], ps, Relu,
                                             bias=b1_t[:, hc, :])

                # --- layer 2 + v-dot (ACT -> DVE accumulate) ---
                facc = [fbp.tile((P, NT), f32, name=f"facc{n}", tag="facc")
                        for n in range(SN)]
                for fc in range(HC):
                    w2s = w2p.tile((P, HC, P), fp16, name="w2s", tag="w2s")
                    nc.sync.dma_start(w2s[:], W2d[fc])
                    for n in range(SN):
                        ns = slice(n * NT, (n + 1) * NT)
                        ps = mmps.tile((P, NT), f32, name="ps_2", tag="mm")
                        for hc in range(HC):
                            nc.tensor.matmul(ps, w2s[:, hc, :], h1[:, hc, ns],
                                             start=(hc == 0),
                                             stop=(hc == HC - 1))
                        h2t = h2p.tile((P, NT), fp16, name="h2t", tag="h2t")
                        nc.scalar.activation(h2t[:], ps, Relu,
                                             bias=b2_t[:, fc, :])
                        if fc == 0:
                            nc.vector.tensor_scalar_mul(facc[n][:], h2t[:],
                                                        v_t[:, fc, :])
                        else:
                            tt = tp.tile((P, NT), f32, name="tt", tag="tt")
                            nc.vector.tensor_scalar_mul(tt[:], h2t[:],
                                                        v_t[:, fc, :])
                            nc.vector.tensor_tensor(facc[n][:], facc[n][:],
                                                    tt[:], Add)
                # partition-reduce facc via a single fp32 ones-matvec per n
                for n in range(SN):
                    sps = vps.tile((1, NT), f32, name="sps", tag="vec")
                    nc.tensor.matmul(sps, ones_t[:], facc[n][:],
                                     start=True, stop=True)
                    ot = outp.tile((1, NT), f32, name="ot", tag="ot")
                    nc.vector.tensor_copy(ot[:], sps)
                    nc.sync.dma_start(
                        out_s[0:1, s0 + n * NT:s0 + (n + 1) * NT], ot[:])
    nc.compile()
    return nc


_STATE = {}
LAST_RESULTS = None  # BassKernelResults of the most recent device run
LAST_RUN_SECONDS = None  # wall time of the device-run call (excl. host prep)


def _get_nc(cfg):
    key = (cfg.B, cfg.D, cfg.H, cfg.GH, cfg.NT, cfg.SLAB, cfg.reps)
    if key not in _STATE:
        _STATE[key] = _build_nc(cfg)
    return _STATE[key]


def _fold(W3, b3, A, Wo):
    """v_k = W3_k @ (B_k B_k^T Wo),  c_k = b3_k . (B_k B_k^T Wo)  in float64."""
    A64 = A.astype(np.float64)
    S = A64 - A64.T
    I = np.eye(A.shape[0])
    Q = np.linalg.solve(I - S, I + S)
    K = W3.shape[0]
    sub = Q.shape[1] // K
    Bq = Q.reshape(Q.shape[0], K, sub)                      # [d, k, s]
    coef = np.einsum('dks,d->ks', Bq, Wo[:, 0].astype(np.float64))
    w = np.einsum('dks,ks->kd', Bq, coef)                   # (K, dim)
    v = np.einsum('kfd,kd->kf', W3.astype(np.float64), w)   # (K, H2)
    c = np.einsum('kd,kd->k', b3.astype(np.float64), w)     # (K,)
    return v, c


def _prep_in_maps(cfg, x, W1, b1, W2, b2, v, Wg1, bg1, Wg2, bg2):
    fp16 = np.float16
    f32 = np.float32
    K = W1.shape[0]
    DC, HC, GC, GB = cfg.DC, cfg.HC, cfg.GC, cfg.GB
    NE = K

    # [p, dc, b]
    xT = np.ascontiguousarray(
        x.astype(fp16).T.reshape(DC, P, cfg.B).transpose(1, 0, 2))
    W1p = np.ascontiguousarray(
        W1.astype(fp16).reshape(K, DC, P, HC, P).transpose(0, 3, 2, 1, 4))
    W2p = np.ascontiguousarray(
        W2.astype(fp16).reshape(K, HC, P, HC, P).transpose(0, 3, 2, 1, 4))
    Wg1p = np.ascontiguousarray(
        Wg1.astype(fp16).reshape(DC, P, cfg.GH).transpose(1, 0, 2))

    # packed f32 consts (P, 3*HC+GC, 1): [b1 | b2 | bg1 | v]
    NF = 3 * HC + GC
    constf = np.empty((K, P, NF, 1), f32)
    constf[:, :, 0:HC, 0] = b1.astype(f32).reshape(K, HC, P).transpose(0, 2, 1)
    constf[:, :, HC:2 * HC, 0] = (
        b2.astype(f32).reshape(K, HC, P).transpose(0, 2, 1))
    constf[:, :, 2 * HC:2 * HC + GC, 0] = (
        bg1.astype(f32).reshape(GC, P).T[None])
    constf[:, :, 2 * HC + GC:, 0] = (
        v.astype(f32).reshape(K, HC, P).transpose(0, 2, 1))
    # packed fp16 consts (P, GC*NE, 1): [Wg2 all columns]
    NH = GC * NE
    consth = np.empty((K, P, NH, 1), fp16)
    # [p, gc*NE + e] = Wg2[gc*P+p, e] — same for every core
    wg2_packed = (
        Wg2.astype(fp16).reshape(GC, P, NE).transpose(1, 0, 2).reshape(
            P, GC * NE))
    consth[:, :, :, 0] = wg2_packed[None]

    in_maps = []
    for k in range(K):
        in_maps.append({
            "xT": xT,
            "xG": np.ascontiguousarray(xT[:, :, k * GB:(k + 1) * GB]),
            "W1": W1p[k],
            "W2": W2p[k],
            "constf": constf[k],
            "consth": consth[k],
            "Wg1": Wg1p,
        })
    return in_maps


def kernel(x, W1, b1, W2, b2, W3, b3, Wg1, bg1, Wg2, bg2, A, Wo, bo):
    global LAST_RESULTS, LAST_RUN_SECONDS
    import time

    from concourse.bass_utils import run_bass_kernel_spmd

    cfg = _Cfg(B=x.shape[0], D=x.shape[1], H=W1.shape[2], GH=Wg1.shape[1])
    K = W1.shape[0]

    v, c = _fold(W3, b3, A, Wo)
    in_maps = _prep_in_maps(cfg, x, W1, b1, W2, b2, v, Wg1, bg1, Wg2, bg2)
    nc = _get_nc(cfg)

    trace = bool(int(os.environ.get("MOE_TRACE", "0")))
    t0 = time.time()
    res = run_bass_kernel_spmd(
        nc, in_maps, core_ids=list(range(K)), trace=trace,
        trace_cores=list(range(K)) if trace else None,
    )
    LAST_RUN_SECONDS = time.time() - t0
    LAST_RESULTS = res

    GB = cfg.GB
    s = np.stack([r["out_s"][0] for r in res.results]).astype(np.float64)
    # logits: core k supplies all-expert logits for batch slice k
    logit = np.concatenate([r["out_e"] for r in res.results],
                           axis=1).astype(np.float64)
    e = np.exp(logit + bg2.astype(np.float64)[:, None])
    num = (e * (s + c[:, None])).sum(axis=0)
    den = e.sum(axis=0)
    out = num / den + float(bo[0])
    return out.astype(np.float32)[:, None]


# revision 11
# speedup vs baseline: 6.6701x; 1.0797x over previous
# Trainium2 Bass kernel for nn_MixtureOfExperts_37237366456694.
#
# Reference computation (B=4096, D=1024, H1=H2=4096, D_OUT=1024, K=8, G_H=512):
#   U[:,k,:] = MLP_k(x)                      (3-layer ReLU MLP per expert)
#   g        = softmax(gate_MLP(x))          (B, K)
#   Q        = cayley(A); B_k = Q[:, k*128:(k+1)*128]
#   V[:,k,:] = U[:,k,:] @ (B_k B_k^T)
#   out      = (sum_k g[:,k] * V[:,k,:]) @ Wo + bo
#
# Key algebraic collapse (exact):
#   out[b] = sum_k g[b,k] * (U[b,k,:] @ w_k) + bo,   w_k = B_k B_k^T Wo
#          = sum_k g[b,k] * (h2_k[b] @ v_k + c_k) + bo
#   with v_k = W3_k @ w_k  (H2-vector), c_k = b3_k . w_k  (scalar).
# So the third expert layer + subspace projection + output head reduce to a
# dot against a precomputed vector.  The tiny Cayley solve / folds are done
# on host in float64; the heavy compute (two 4096-wide matmul layers) runs
# on device in fp16 with f32 PSUM accumulation.
#
# Sharding: expert-parallel — core k owns expert k (its W1/W2/b1/b2/v shards).
# The kernel is PE(tensor-engine)-row-bound, so everything not a dense matmul
# is pushed off the PE:
#   * v-dot: ACT evicts h2 = relu(psum+b2); DVE (idle) accumulates
#     facc += h2*v in f32 with per-partition signed v across the fc chunks;
#     one 512-row fp32 PE matvec against a ones vector per (slab,n) reduces
#     facc over partitions at slab end (~7us/rep on the PE instead of the
#     ~66us/rep of per-chunk PE matvecs).
#   * gate: batch-sharded across the 8 cores — core k computes the gate
#     hidden layer + ALL-expert logits for its 1/8 slice of the batch only
#     (the host combines, so no collective is needed).  Replaces an 8x
#     replicated full-batch gate.
# Host combines:  out = (sum_k e_k*(s_k + c_k)) / (sum_k e_k) + bo  with
# e_k = exp(logit_k + bg2_k) in float64 — the softmax normalizer is the
# cross-expert sum, i.e. the "all-reduce" term, done on host for free.
import os

import numpy as np

P = 128


class _Cfg:
    def __init__(self, B=4096, D=1024, H=4096, GH=512, NT=512, SLAB=1024,
                 reps=1, bf16=False):
        self.B, self.D, self.H, self.GH, self.NT, self.SLAB = B, D, H, GH, NT, SLAB
        self.DC = D // P      # d_in chunks
        self.HC = H // P      # hidden chunks (H1 == H2)
        self.GC = GH // P     # gate hidden chunks
        self.NSLAB = B // SLAB
        self.SN = SLAB // NT  # n-tiles per slab
        self.GB = B // 8      # per-core gate batch slice
        self.reps = reps      # >1 only for differential benchmarking
        self.bf16 = bf16      # 16-bit dtype for weights/activations


def _build_nc(cfg):
    import contextlib

    import concourse.bass as bass  # noqa: F401
    import concourse.mybir as mybir
    import concourse.tile as tile
    from concourse import bacc

    fp16 = mybir.dt.bfloat16 if cfg.bf16 else mybir.dt.float16
    f32 = mybir.dt.float32
    Relu = mybir.ActivationFunctionType.Relu
    Add = mybir.AluOpType.add

    B, DC, HC, GC, NT, SLAB, SN, NSLAB, GB = (
        cfg.B, cfg.DC, cfg.HC, cfg.GC, cfg.NT, cfg.SLAB, cfg.SN, cfg.NSLAB,
        cfg.GB)
    GH = cfg.GH
    NE = 8  # experts (== cores)

    nc = bacc.Bacc(None, target_bir_lowering=False)
    # [p, dc, b] = x[b, dc*P+p]
    xTd = nc.dram_tensor("xT", (P, DC, B), fp16, kind="ExternalInput")
    # per-core gate slice: [p, dc, b'] = x[core*GB + b', dc*P+p]
    xGd = nc.dram_tensor("xG", (P, DC, GB), fp16, kind="ExternalInput")
    # [hc, p, dc, m] = W1[dc*P+p, hc*P+m]
    W1d = nc.dram_tensor("W1", (HC, P, DC, P), fp16, kind="ExternalInput")
    # [fc, p, hc, m] = W2[hc*P+p, fc*P+m]
    W2d = nc.dram_tensor("W2", (HC, P, HC, P), fp16, kind="ExternalInput")
    # f32 consts: [b1 (HC) | b2 (HC) | bg1 (GC) | v (HC)]
    NF = 3 * HC + GC
    cfd = nc.dram_tensor("constf", (P, NF, 1), f32, kind="ExternalInput")
    # fp16 consts: [Wg2 all-expert columns (GC x NE)]
    NH = GC * NE
    chd = nc.dram_tensor("consth", (P, NH, 1), fp16, kind="ExternalInput")
    # [p, dc, gh] = Wg1[dc*P+p, gh]
    Wg1d = nc.dram_tensor("Wg1", (P, DC, GH), fp16, kind="ExternalInput")
    out_s = nc.dram_tensor("out_s", (1, B), f32, kind="ExternalOutput")
    # raw gate logits for ALL experts, this core's batch slice
    out_e = nc.dram_tensor("out_e", (NE, GB), f32, kind="ExternalOutput")

    with tile.TileContext(nc) as tc:
        with (
            tc.tile_pool(name="const", bufs=1) as const,
            tc.tile_pool(name="xp", bufs=2) as xp,
            tc.tile_pool(name="zp", bufs=1) as zp,
            tc.tile_pool(name="w1p", bufs=4) as w1p,
            tc.tile_pool(name="w2p", bufs=5) as w2p,
            tc.tile_pool(name="h1p", bufs=1) as h1p,
            tc.tile_pool(name="h2p", bufs=4) as h2p,
            tc.tile_pool(name="tp", bufs=3) as tp,
            tc.tile_pool(name="fbp", bufs=SN + 2) as fbp,
            tc.tile_pool(name="outp", bufs=4) as outp,
            tc.tile_pool(name="mmps", bufs=6, space="PSUM") as mmps,
            tc.tile_pool(name="vps", bufs=2, space="PSUM") as vps,
        ):
            # --- constants resident in SBUF for the whole kernel ---
            wg1_t = const.tile((P, DC, GH), fp16)
            nc.sync.dma_start(wg1_t[:], Wg1d[:])
            xg_t = const.tile((P, DC, GB), fp16)
            nc.sync.dma_start(xg_t[:], xGd[:])
            cf_t = const.tile((P, NF, 1), f32)
            nc.sync.dma_start(cf_t[:], cfd[:])
            ch_t = const.tile((P, NH, 1), fp16)
            nc.sync.dma_start(ch_t[:], chd[:])
            ones_t = const.tile((P, 1), f32)
            nc.vector.memset(ones_t[:], 1.0)
            b1_t = cf_t[:, 0:HC, :]
            b2_t = cf_t[:, HC:2 * HC, :]
            bg1_t = cf_t[:, 2 * HC:2 * HC + GC, :]
            v_t = cf_t[:, 2 * HC + GC:3 * HC + GC, :]

            # reps>1: wrap the computation in a hardware loop.  Every
            # iteration is instruction-identical (same addresses), so this
            # multiplies device time by reps while keeping the NEFF small —
            # used by test.py for differential timing.
            loop_cm = tc.For_i(0, cfg.reps) if cfg.reps > 1 else (
                contextlib.nullcontext())
            with loop_cm:
              # --- gate MLP on this core's batch slice, all experts ---
              GN = GB // NT
              z1 = zp.tile((P, GC, GB), fp16, name="z1", tag="z1")
              for gc in range(GC):
                  for gn in range(GN):
                      gs = slice(gn * NT, (gn + 1) * NT)
                      ps = mmps.tile((P, NT), f32, name="ps_g", tag="mm")
                      for dc in range(DC):
                          nc.tensor.matmul(
                              ps, wg1_t[:, dc, gc * P:(gc + 1) * P],
                              xg_t[:, dc, gs],
                              start=(dc == 0), stop=(dc == DC - 1))
                      nc.scalar.activation(z1[:, gc, gs], ps, Relu,
                                           bias=bg1_t[:, gc, :])
              for gn in range(GN):
                  gs = slice(gn * NT, (gn + 1) * NT)
                  lg = vps.tile((NE, NT), f32, name="lg", tag="vec")
                  for gc in range(GC):
                      nc.tensor.matmul(
                          lg, ch_t[:, gc * NE:(gc + 1) * NE, :],
                          z1[:, gc, gs],
                          start=(gc == 0), stop=(gc == GC - 1))
                  lt = outp.tile((NE, NT), f32, name="lt", tag="lt")
                  nc.vector.tensor_copy(lt[:], lg)
                  nc.sync.dma_start(out_e[:, gs], lt[:])

              for sl in range(NSLAB):
                s0 = sl * SLAB
                # --- x slab (transposed: d on partitions) ---
                xt = xp.tile((P, DC, SLAB), fp16, name="xt", tag="xt")
                nc.sync.dma_start(xt[:], xTd[:, :, s0:s0 + SLAB])

                # --- layer 1: h1 = relu(x @ W1 + b1), stored transposed ---
                h1 = h1p.tile((P, HC, SLAB), fp16, name="h1", tag="h1")
                for hc in range(HC):
                    w1s = w1p.tile((P, DC, P), fp16, name="w1s", tag="w1s")
                    nc.sync.dma_start(w1s[:], W1d[hc])
                    for n in range(SN):
                        ns = slice(n * NT, (n + 1) * NT)
                        ps = mmps.tile((P, NT), f32, name="ps_1", tag="mm")
                        for dc in range(DC):
                            nc.tensor.matmul(ps, w1s[:, dc, :], xt[:, dc, ns],
                                             start=(dc == 0),
                                             stop=(dc == DC - 1))
                        nc.scalar.activation(h1[:, hc, ns], ps, Relu,
                                             bias=b1_t[:, hc, :])

                # --- layer 2 + v-dot (ACT -> DVE accumulate) ---
                facc = [fbp.tile((P, NT), f32, name=f"facc{n}", tag="facc")
                        for n in range(SN)]
                for fc in range(HC):
                    w2s = w2p.tile((P, HC, P), fp16, name="w2s", tag="w2s")
                    nc.sync.dma_start(w2s[:], W2d[fc])
                    for n in range(SN):
                        ns = slice(n * NT, (n + 1) * NT)
                        ps = mmps.tile((P, NT), f32, name="ps_2", tag="mm")
                        for hc in range(HC):
                            nc.tensor.matmul(ps, w2s[:, hc, :], h1[:, hc, ns],
                                             start=(hc == 0),
                                             stop=(hc == HC - 1))
                        h2t = h2p.tile((P, NT), fp16, name="h2t", tag="h2t")
                        nc.scalar.activation(h2t[:], ps, Relu,
                                             bias=b2_t[:, fc, :])
                        if fc == 0:
                            nc.vector.tensor_scalar_mul(facc[n][:], h2t[:],
                                                        v_t[:, fc, :])
                        else:
                            tt = tp.tile((P, NT), f32, name="tt", tag="tt")
                            nc.vector.tensor_scalar_mul(tt[:], h2t[:],
                                                        v_t[:, fc, :])
                            nc.vector.tensor_tensor(facc[n][:], facc[n][:],
                                                    tt[:], Add)
                # partition-reduce facc via a single fp32 ones-matvec per n
                for n in range(SN):
                    sps = vps.tile((1, NT), f32, name="sps", tag="vec")
                    nc.tensor.matmul(sps, ones_t[:], facc[n][:],
                                     start=True, stop=True)
                    ot = outp.tile((1, NT), f32, name="ot", tag="ot")
                    nc.vector.tensor_copy(ot[:], sps)
                    nc.sync.dma_start(
                        out_s[0:1, s0 + n * NT:s0 + (n + 1) * NT], ot[:])
    nc.compile()
    return nc


_STATE = {}
LAST_RESULTS = None  # BassKernelResults of the most recent device run
LAST_RUN_SECONDS = None  # wall time of the device-run call (excl. host prep)


def _get_nc(cfg):
    key = (cfg.B, cfg.D, cfg.H, cfg.GH, cfg.NT, cfg.SLAB, cfg.reps,
           cfg.bf16)
    if key not in _STATE:
        _STATE[key] = _build_nc(cfg)
    return _STATE[key]


def _fold(W3, b3, A, Wo):
    """v_k = W3_k @ (B_k B_k^T Wo),  c_k = b3_k . (B_k B_k^T Wo)  in float64."""
    A64 = A.astype(np.float64)
    S = A64 - A64.T
    I = np.eye(A.shape[0])
    Q = np.linalg.solve(I - S, I + S)
    K = W3.shape[0]
    sub = Q.shape[1] // K
    Bq = Q.reshape(Q.shape[0], K, sub)                      # [d, k, s]
    coef = np.einsum('dks,d->ks', Bq, Wo[:, 0].astype(np.float64))
    w = np.einsum('dks,ks->kd', Bq, coef)                   # (K, dim)
    v = np.einsum('kfd,kd->kf', W3.astype(np.float64), w)   # (K, H2)
    c = np.einsum('kd,kd->k', b3.astype(np.float64), w)     # (K,)
    return v, c


def _prep_in_maps(cfg, x, W1, b1, W2, b2, v, Wg1, bg1, Wg2, bg2):
    if cfg.bf16:
        import ml_dtypes
        fp16 = ml_dtypes.bfloat16
    else:
        fp16 = np.float16
    f32 = np.float32
    K = W1.shape[0]
    DC, HC, GC, GB = cfg.DC, cfg.HC, cfg.GC, cfg.GB
    NE = K

    # [p, dc, b]
    xT = np.ascontiguousarray(
        x.astype(fp16).T.reshape(DC, P, cfg.B).transpose(1, 0, 2))
    W1p = np.ascontiguousarray(
        W1.astype(fp16).reshape(K, DC, P, HC, P).transpose(0, 3, 2, 1, 4))
    W2p = np.ascontiguousarray(
        W2.astype(fp16).reshape(K, HC, P, HC, P).transpose(0, 3, 2, 1, 4))
    Wg1p = np.ascontiguousarray(
        Wg1.astype(fp16).reshape(DC, P, cfg.GH).transpose(1, 0, 2))

    # packed f32 consts (P, 3*HC+GC, 1): [b1 | b2 | bg1 | v]
    NF = 3 * HC + GC
    constf = np.empty((K, P, NF, 1), f32)
    constf[:, :, 0:HC, 0] = b1.astype(f32).reshape(K, HC, P).transpose(0, 2, 1)
    constf[:, :, HC:2 * HC, 0] = (
        b2.astype(f32).reshape(K, HC, P).transpose(0, 2, 1))
    constf[:, :, 2 * HC:2 * HC + GC, 0] = (
        bg1.astype(f32).reshape(GC, P).T[None])
    constf[:, :, 2 * HC + GC:, 0] = (
        v.astype(f32).reshape(K, HC, P).transpose(0, 2, 1))
    # packed fp16 consts (P, GC*NE, 1): [Wg2 all columns]
    NH = GC * NE
    consth = np.empty((K, P, NH, 1), fp16)
    # [p, gc*NE + e] = Wg2[gc*P+p, e] — same for every core
    wg2_packed = (
        Wg2.astype(fp16).reshape(GC, P, NE).transpose(1, 0, 2).reshape(
            P, GC * NE))
    consth[:, :, :, 0] = wg2_packed[None]

    in_maps = []
    for k in range(K):
        in_maps.append({
            "xT": xT,
            "xG": np.ascontiguousarray(xT[:, :, k * GB:(k + 1) * GB]),
            "W1": W1p[k],
            "W2": W2p[k],
            "constf": constf[k],
            "consth": consth[k],
            "Wg1": Wg1p,
        })
    return in_maps


def kernel(x, W1, b1, W2, b2, W3, b3, Wg1, bg1, Wg2, bg2, A, Wo, bo):
    global LAST_RESULTS, LAST_RUN_SECONDS
    import time

    from concourse.bass_utils import run_bass_kernel_spmd

    cfg = _Cfg(B=x.shape[0], D=x.shape[1], H=W1.shape[2], GH=Wg1.shape[1])
    K = W1.shape[0]

    v, c = _fold(W3, b3, A, Wo)
    in_maps = _prep_in_maps(cfg, x, W1, b1, W2, b2, v, Wg1, bg1, Wg2, bg2)
    nc = _get_nc(cfg)

    trace = bool(int(os.environ.get("MOE_TRACE", "0")))
    t0 = time.time()
    res = run_bass_kernel_spmd(
        nc, in_maps, core_ids=list(range(K)), trace=trace,
        trace_cores=list(range(K)) if trace else None,
    )
    LAST_RUN_SECONDS = time.time() - t0
    LAST_RESULTS = res

    GB = cfg.GB
    s = np.stack([r["out_s"][0] for r in res.results]).astype(np.float64)
    # logits: core k supplies all-expert logits for batch slice k
    logit = np.concatenate([r["out_e"] for r in res.results],
                           axis=1).astype(np.float64)
    e = np.exp(logit + bg2.astype(np.float64)[:, None])
    num = (e * (s + c[:, None])).sum(axis=0)
    den = e.sum(axis=0)
    out = num / den + float(bo[0])
    return out.astype(np.float32)[:, None]
